# revision 44
# baseline (speedup 1.0000x reference)
"""Distributed multi-head attention kernel for 8 TRN2 NeuronCores.

Problem: x [4, 2048, 1024] -> qkv proj -> 16-head attention (d=64)
         -> out proj + bias -> [4, 2048, 1024].

Sharding (no collectives): core i handles batch b = i//2 and query-half
half = i%2 (1024 query tokens). Each core computes K/V for its batch's
full 2048-token sequence (duplicated within the pair of cores sharing a
batch) and Q only for its own 1024 tokens. The host rotates the token
axis per core so the core's query tokens are always tokens [0, 1024) of
its input -- attention is permutation-invariant over keys, so K/V token
order does not matter.

Per-core pipeline (everything bf16 on the TensorE, fp32 PSUM accum):
  proj:  Q^T [d, q] / K^T [d, k] head-pairs packed on 128 partitions;
         V [k, d] in 65-wide per-head blocks with a ones column
         (the PV matmul then yields softmax denominators for free).
  attn:  per head: S^T = K @ Q^T -> exp on ScalarE (x0.125 fused, no
         max subtraction; scores are O(1) by construction) -> bf16 P^T
         -> PV accumulation U^T[65, q]; row 64 = denominator. S(k)/exp
         are emitted one k-step ahead of PV(k-1) so the in-order PE
         queue never head-of-line blocks on the exp.
         Tail: one DVE copy [65, q] frees the PSUM slot; the pair tile
         [128, q] bf16 is filled off the PSUM chain (GpSimd lane-local
         for rows 0:64, SBUF-to-SBUF DMA for the cross-partition odd
         half). The slow fp16 softmax reciprocal is split into 4
         [1, 256] chunks drained one per k-step of the NEXT head, so
         the in-order DVE queue never carries a block the PE could
         stall behind (via the mm_psum bank rotation). Normalize =
         ones-row broadcast matmul + DVE multiply straight against the
         PSUM broadcast (no SBUF staging), one head late.
  out:   head-pair packed: stationary [128(d of 2 heads), 128(q)] at
         K=128 keeps the PE at full clock -- K=64 accumulation chains
         measured ~2x slower (p-state/row-group effect). Pass A (pairs
         0-3 + bias) fills PE gaps during the attention tail; pass B
         (pairs 7,4,5,6) accumulates onto resident partials -> out.

The two halves' projections and attention phases are arranged so the
PE always has matmul work while the ScalarE grinds through exp()
(~285us of ACT vs ~465us of PE busy; the kernel is PE-bound).
"""

import numpy as np
import ml_dtypes

B = 4
N = 2048
DIM = 1024
HEADS = 16
DH = 64
NQ = 1024  # query tokens per core
NCORES = 8
NPAIR = HEADS // 2

_CACHE = {}


def _build_nc():
    from contextlib import ExitStack

    import concourse.bass as bass
    import concourse.mybir as mybir
    import concourse.tile as tile
    from concourse import bacc

    f32 = mybir.dt.float32
    bf16 = mybir.dt.bfloat16
    f16 = mybir.dt.float16
    EXP = mybir.ActivationFunctionType.Exp

    nc = bacc.Bacc("TRN2", target_bir_lowering=False, debug=False,
                   num_devices=NCORES)

    xt_d = nc.dram_tensor("xt", [DIM, N], bf16, kind="ExternalInput")
    wqkv_d = nc.dram_tensor("wqkv", [DIM, 3 * DIM], bf16, kind="ExternalInput")
    wo_d = nc.dram_tensor("wo", [HEADS, DH, DIM], bf16, kind="ExternalInput")
    brow_d = nc.dram_tensor("brow", [1, DIM], bf16, kind="ExternalInput")
    out_d = nc.dram_tensor("out", [NQ, DIM], f32, kind="ExternalOutput")

    with tile.TileContext(nc) as tc, ExitStack() as top:
        const_pool = top.enter_context(tc.tile_pool(name="const", bufs=1))
        mm_psum = top.enter_context(tc.tile_pool(name="mmps", bufs=2, space="PSUM"))
        sp_psum = top.enter_context(tc.tile_pool(name="spps", bufs=2, space="PSUM"))
        u_psum = top.enter_context(tc.tile_pool(name="ups", bufs=1, space="PSUM"))
        es_pool = top.enter_context(tc.tile_pool(name="es", bufs=6))
        dn_pool = top.enter_context(tc.tile_pool(name="dn", bufs=4))
        rec_pool = top.enter_context(tc.tile_pool(name="rec", bufs=4))
        bc_pool = top.enter_context(tc.tile_pool(name="bc", bufs=6))
        uraw_a = top.enter_context(tc.tile_pool(name="uraw_a", bufs=1))

        brow_t = const_pool.tile([1, DIM], bf16, tag="brow", name="brow")
        nc.sync.dma_start(brow_t[:], brow_d.ap()[:])
        ones_t = const_pool.tile([1, 128], bf16, tag="ones", name="ones")
        nc.gpsimd.memset(ones_t[:], 1.0)
        # all-ones f16 readable at partition base 64 (row 64) so the
        # denom-broadcast matmul's operands share a base partition
        ones_bc = const_pool.tile([65, 128], f16, tag="ones_bc",
                                  name="ones_bc")
        nc.gpsimd.memset(ones_bc[:], 1.0)

        pairU = [None] * NPAIR  # [128, NQ] bf16: head 2p rows 0:64, 2p+1 rows 64:128
        urs = {}                # per-head [65, NQ] bf16 raw U + denom row
        _recs = {}              # per-head [65, NQ] f16 denom reciprocal (row 64)

        def proj_units(half, w_pool, xtA, xtB, QT, KT, VT):
            """Emission closures, one PSUM-group each.

            Order: all of V, then K/Q alternating per head-pair chunk so
            early head pairs become ready as soon as possible.
            """
            def dma_factory(col0):
                box = [None]
                def dma():
                    if box[0] is None:
                        wb = [w_pool.tile([128, 512], bf16, tag=f"w{fc}",
                                          name=f"w{fc}") for fc in range(8)]
                        for fc in range(8):
                            nc.sync.dma_start(
                                wb[fc][:],
                                wqkv_d.ap()[fc * 128:(fc + 1) * 128,
                                            col0:col0 + 512])
                        box[0] = wb
                    return box[0]
                return dma

            dma_v = dma_factory(2 * DIM + half * 512)
            dma_k = dma_factory(DIM + half * 512)
            dma_q = dma_factory(half * 512)

            def v_unit(mk, dma=dma_v):
                wb = dma()
                xth = xtA if mk < 8 else xtB
                c0 = (mk % 8) * 128
                ps = mm_psum.tile([128, 512], f32, tag="mm", name="mm")
                for fc in range(8):
                    nc.tensor.matmul(
                        ps[:], xth[fc][:, c0:c0 + 128], wb[fc][:],
                        start=(fc == 0), stop=(fc == 7))
                nc.vector.tensor_copy(
                    VT[mk][:, :, 0:64],
                    ps[:].rearrange("p (h d) -> p h d", d=64))
                nc.gpsimd.memset(VT[mk][:, :, 64:65], 1.0)

            def qk_unit(dma, dest, m4, t):
                wb = dma()
                xth = xtA if t < 2 else xtB
                c0 = (t % 2) * 512
                ps = mm_psum.tile([128, 512], f32, tag="mm", name="mm")
                for fc in range(8):
                    nc.tensor.matmul(
                        ps[:], wb[fc][:, m4 * 128:(m4 + 1) * 128],
                        xth[fc][:, c0:c0 + 512],
                        start=(fc == 0), stop=(fc == 7))
                nc.vector.tensor_copy(
                    dest[m4][:, t * 512:(t + 1) * 512], ps[:])

            units = [lambda mk=mk: v_unit(mk) for mk in range(16)]
            for m4 in range(4):
                for t in range(4):
                    units.append(lambda m4=m4, t=t: qk_unit(dma_k, KT, m4, t))
                for t in range(2):
                    units.append(lambda m4=m4, t=t: qk_unit(dma_q, QT, m4, t))
            return units, (dma_v, dma_k, dma_q)

        def emit_norm(p):
            """Normalize pair p's raw U rows by softmax denominators."""
            for par in range(2):
                h = 2 * p + par
                r0 = par * 64
                rec = _recs.pop(h)
                for qc in range(2):
                    bc = mm_psum.tile([128, 512], f32, tag="mm", name="bc")
                    nc.tensor.matmul(
                        bc[:], ones_bc[64:65, :],
                        rec[64:65, qc * 512:(qc + 1) * 512],
                        start=True, stop=True)
                    # multiply straight against the PSUM broadcast (same
                    # base partition on both inputs); no SBUF staging
                    nc.vector.tensor_mul(
                        pairU[p][r0:r0 + 64, qc * 512:(qc + 1) * 512],
                        pairU[p][r0:r0 + 64, qc * 512:(qc + 1) * 512],
                        bc[r0:r0 + 64, :])

        def emit_attn(heads, QTs, KTs, VTs, pairU_pool, fillers,
                      hooks=None):
            """Attention for the given heads; filler units spread across
            k-steps. Normalize for pair p is emitted after head 2p+3."""
            fillers = list(fillers)
            nfill = len(fillers)
            pending_dve = state.setdefault("pending_dve", [])
            steps = len(heads) * 16
            done = 0
            for hh_i, h in enumerate(heads):
                if hooks and h in hooks:
                    hooks[h]()
                half = h // 8
                hh = h % 8
                par = h % 2
                p = h // 2
                QT, KT, VT = QTs[half], KTs[half], VTs[half]
                pair = hh // 2
                hb = (hh % 2) * 64
                Ups = u_psum.tile([65, 2, 512], f32, tag="up", name="up")
                ess = [None, None]
                drain_dve = list(pending_dve)
                pending_dve.clear()
                # S(k)/exp(k) are emitted one step ahead of PV(k-1) so
                # the PE queue never head-of-line blocks on the exp and
                # the new head's first PV gets PSUM-release slack.
                for k in range(16):
                    sp = sp_psum.tile([128, 2, 512], f32, tag="sp", name="sp")
                    for qc in range(2):
                        nc.tensor.matmul(
                            sp[:, qc, :],
                            KT[pair][hb:hb + 64, k * 128:(k + 1) * 128],
                            QT[pair][hb:hb + 64, qc * 512:(qc + 1) * 512],
                            start=True, stop=True)
                    es = es_pool.tile([128, 2, 512], bf16, tag="es", name="es")
                    nc.scalar.activation(es[:], sp[:], EXP, scale=0.125)
                    ess[k % 2] = es
                    if k >= 1:
                        for qc in range(2):
                            nc.tensor.matmul(
                                Ups[:, qc, :],
                                VT[k - 1][:, hh, :],
                                ess[(k - 1) % 2][:, qc, :],
                                start=(k == 1), stop=False)
                    if drain_dve and k % 2 == 1:
                        drain_dve.pop(0)()
                    done += 1
                    while fillers and (nfill - len(fillers)) * steps < done * nfill:
                        fillers.pop(0)()
                for qc in range(2):
                    nc.tensor.matmul(
                        Ups[:, qc, :], VT[15][:, hh, :], ess[15 % 2][:, qc, :],
                        start=False, stop=True)
                # free the PSUM slot fast: U rows on DVE, denom row on
                # GpSimd (parallel engines)
                if par == 0:
                    pairU[p] = pairU_pool(p).tile(
                        [128, NQ], bf16, tag=f"pu{p}", name=f"pu{p}")
                # one fast DVE copy frees the PSUM slot; the pair-packed
                # re-copy runs off the PSUM chain (GpSimd lane-local for
                # the even half, DMA for the cross-partition odd half)
                ur = dn_pool.tile([65, NQ], bf16, tag="ur", name=f"ur{h}")
                urs[h] = ur
                nc.vector.tensor_copy(
                    ur[:].rearrange("p (a b) -> p a b", a=2),
                    Ups[:, :, :])
                if par == 0:
                    nc.gpsimd.tensor_copy(pairU[p][0:64, :], ur[0:64, :])
                else:
                    nc.sync.dma_start(pairU[p][64:128, :], ur[0:64, :])
                # the slow reciprocal is queued in ~1.6us chunks drained
                # one per k-step of the NEXT head, so the in-order DVE
                # queue never carries a block the PE could stall behind
                rec = rec_pool.tile([65, NQ], f16, tag="rec", name="rec")
                _recs[h] = rec

                def recip_chunk(ci, rec=rec, ur=ur):
                    with nc.allow_low_precision(reason="softmax recip f16"):
                        nc.vector.reciprocal(
                            rec[64:65, ci * 256:(ci + 1) * 256],
                            ur[64:65, ci * 256:(ci + 1) * 256])
                for ci in range(4):
                    pending_dve.append(lambda ci=ci, f=recip_chunk: f(ci))
                # pair norm, emitted one head late so the recip chain is
                # long done before the PE broadcast matmul queues
                if par == 1 and p > 0:
                    emit_norm(p - 1)
            for f in fillers:
                f()

        # ---------------- emission ----------------
        if True:
            xt_pool = tc.alloc_tile_pool(name="xt", bufs=1)
            w_pool = tc.alloc_tile_pool(name="w", bufs=3)
            xtA = [xt_pool.tile([128, NQ], bf16, tag=f"xa{i}", name=f"xa{i}")
                   for i in range(8)]
            xtB = [xt_pool.tile([128, NQ], bf16, tag=f"xb{i}", name=f"xb{i}")
                   for i in range(8)]

            qkv0 = tc.alloc_tile_pool(name="qkv0", bufs=1)
            QT0 = [qkv0.tile([128, NQ], bf16, tag=f"q{m}", name=f"q0{m}")
                   for m in range(4)]
            KT0 = [qkv0.tile([128, N], bf16, tag=f"k{m}", name=f"k0{m}")
                   for m in range(4)]
            VT0 = [qkv0.tile([128, 8, 65], bf16, tag=f"v{mk}", name=f"v0{mk}")
                   for mk in range(16)]
            p0_units, p0_dmas = proj_units(0, w_pool, xtA, xtB, QT0, KT0, VT0)

            # DMA order tuned for earliest first matmul: wk0 block plus
            # the first 512 token columns of xt half A unblock the K t=0
            # units at ~2MB of input traffic; the rest streams behind.
            p0_dmas[1]()  # wk half 0
            for i in range(8):
                nc.sync.dma_start(xtA[i][:, 0:512],
                                  xt_d.ap()[i * 128:(i + 1) * 128, 0:512])
            p0_dmas[0]()  # wv half 0
            for i in range(8):
                nc.sync.dma_start(xtA[i][:, 512:NQ],
                                  xt_d.ap()[i * 128:(i + 1) * 128, 512:NQ])
            for i in range(8):
                nc.sync.dma_start(xtB[i][:], xt_d.ap()[i * 128:(i + 1) * 128, NQ:N])

            # head-0/1 prerequisites serially (K pair-0 / V / Q pair-0),
            # resequenced to match DMA arrival order
            u = p0_units
            # K(m4, t=0) units need only wk + the first xtA half: they
            # cover the PE from ~12us while the rest of xtA/wv stream in
            prefix = ([u[16], u[22], u[28], u[34]] + u[0:4] + [u[17]] +
                      u[4:8] + [u[18], u[19], u[20], u[21]] + u[8:16])
            for c in prefix:
                c()
            p0_rest = [u[i] for i in range(22, 40) if i not in (22, 28, 34)]

            qkv1 = tc.alloc_tile_pool(name="qkv1", bufs=1, side="right")
            QT1 = [qkv1.tile([128, NQ], bf16, tag=f"q{m}", name=f"q1{m}")
                   for m in range(4)]
            KT1 = [qkv1.tile([128, N], bf16, tag=f"k{m}", name=f"k1{m}")
                   for m in range(4)]
            VT1 = [qkv1.tile([128, 8, 65], bf16, tag=f"v{mk}", name=f"v1{mk}")
                   for mk in range(16)]
            p1_units, p1_dmas = proj_units(1, w_pool, xtA, xtB, QT1, KT1, VT1)

            state = {}

            def setup_b():
                qkv0.release()
                state["uraw_b"] = tc.alloc_tile_pool(name="uraw_b", bufs=1,
                                                     side="right")
                wo_pool = tc.alloc_tile_pool(name="wo", bufs=1, side="right")
                state["wo_pool"] = wo_pool
                WOP = [wo_pool.tile([128, DIM], bf16, tag=f"wo{p}",
                                    name=f"wo{p}") for p in range(NPAIR)]
                state["WOP"] = WOP
                for p in range(NPAIR):
                    nc.sync.dma_start(WOP[p][0:64, :], wo_d.ap()[2 * p])
                    nc.sync.dma_start(WOP[p][64:128, :], wo_d.ap()[2 * p + 1])

            def setup_c():
                # xt and the w-block tiles are dead once proj(1) is done
                w_pool.release()
                xt_pool.release()
                state["st_pool"] = tc.alloc_tile_pool(name="st", bufs=2)
                state["FIN"] = [
                    state["st_pool"].tile([128, DIM], f32, tag=f"fin{qf}",
                                          name=f"fin{qf}", bufs=1)
                    for qf in range(8)]

            # pass A unit: pairs 0-3 + bias for one qf -> resident FIN tile
            def passA(qf):
                WOP = state["WOP"]
                fin = state["FIN"][qf]
                for of in range(2):
                    ps = mm_psum.tile([128, 512], f32, tag="mm", name="mm")
                    for p in range(4):
                        nc.tensor.matmul(
                            ps[:],
                            pairU[p][:, qf * 128:(qf + 1) * 128],
                            WOP[p][:, of * 512:(of + 1) * 512],
                            start=(p == 0), stop=False)
                    nc.tensor.matmul(
                        ps[:], ones_t[:, 0:128],
                        brow_t[:, of * 512:(of + 1) * 512],
                        start=False, stop=True)
                    nc.vector.tensor_copy(fin[:, of * 512:(of + 1) * 512],
                                          ps[:])

            pairU_pool = lambda p: uraw_a if p < 4 else state["uraw_b"]

            # heads 0-9: remaining proj0 + all proj1 units fill PE gaps
            emit_attn(range(0, 10), [QT0, QT1], [KT0, KT1], [VT0, VT1],
                      pairU_pool, p0_rest + p1_units[:34],
                      hooks={8: setup_b})
            # heads 10-15: pass A units fill PE gaps; reserve 3 pass A
            # units to cover the tail norm chain of pair 7
            tail_fill = [lambda qf=qf: passA(qf) for qf in range(8)]
            emit_attn(range(10, 16), [QT0, QT1], [KT0, KT1], [VT0, VT1],
                      pairU_pool,
                      p1_units[34:] + tail_fill[:6],
                      hooks={10: setup_c})
            # interleave head-15's recip chunks with the reserved pass A
            # units so the PE is never exposed to the norm(7) DVE chain
            pend = list(state.get("pending_dve", []))
            state["pending_dve"] = []
            tf = list(tail_fill[6:])
            while pend or tf:
                if tf:
                    tf.pop(0)()
                if pend:
                    pend.pop(0)()
            emit_norm(7)

            # pass B: pairs 7,4,5,6 onto the resident partials -> out.
            # pair 7 (the norm-gated pair) goes FIRST in each accumulation
            # group so the PE pays its normalize wait once, then streams.
            for qf in range(8):
                fin = state["FIN"][qf]
                for of in range(2):
                    ps = mm_psum.tile([128, 512], f32, tag="mm", name="mm")
                    for pi, p in enumerate([7, 4, 5, 6]):
                        nc.tensor.matmul(
                            ps[:],
                            pairU[p][:, qf * 128:(qf + 1) * 128],
                            state["WOP"][p][:, of * 512:(of + 1) * 512],
                            start=(pi == 0), stop=(pi == 3))
                    nc.vector.tensor_add(
                        fin[:, of * 512:(of + 1) * 512],
                        fin[:, of * 512:(of + 1) * 512], ps[:])
                    nc.sync.dma_start(
                        out_d.ap()[qf * 128:(qf + 1) * 128,
                                   of * 512:(of + 1) * 512],
                        fin[:, of * 512:(of + 1) * 512])

            state["st_pool"].release()
            state["wo_pool"].release()
            state["uraw_b"].release()
            qkv1.release()

    nc.compile()
    return nc


def _get_nc():
    if "nc" not in _CACHE:
        _CACHE["nc"] = _build_nc()
    return _CACHE["nc"]


def _make_in_maps(x, w_qkv, w_out, b_out):
    bf = ml_dtypes.bfloat16
    wo = np.ascontiguousarray(w_out.reshape(HEADS, DH, DIM)).astype(bf)
    brow = np.asarray(b_out, np.float32).reshape(1, DIM).astype(bf)
    wqkv = np.ascontiguousarray(w_qkv, np.float32).astype(bf)
    in_maps = []
    for i in range(NCORES):
        b, half = i // 2, i % 2
        xt = np.asarray(x[b], np.float32).T.astype(bf)  # [DIM, N]
        if half:
            xt = np.concatenate([xt[:, NQ:], xt[:, :NQ]], axis=1)
        in_maps.append({
            "xt": np.ascontiguousarray(xt),
            "wqkv": wqkv,
            "wo": wo,
            "brow": brow,
        })
    return in_maps


def _assemble(results):
    out = np.empty((B, N, DIM), np.float32)
    for i in range(NCORES):
        b, half = i // 2, i % 2
        out[b, half * NQ:(half + 1) * NQ, :] = results[i]["out"]
    return out


def run(x, w_qkv, w_out, b_out, trace=False):
    """Run the kernel; returns (output, BassKernelResults)."""
    from concourse.bass_utils import run_bass_kernel_spmd
    nc = _get_nc()
    in_maps = _make_in_maps(x, w_qkv, w_out, b_out)
    res = run_bass_kernel_spmd(nc, in_maps, core_ids=list(range(NCORES)),
                               trace=trace)
    return _assemble(res.results), res


def kernel(x, w_qkv, w_out, b_out):
    out, _ = run(x, w_qkv, w_out, b_out, trace=False)
    return out


# revision 45
# speedup vs baseline: 1.0153x; 1.0153x over previous
"""Distributed multi-head attention kernel for 8 TRN2 NeuronCores.

Problem: x [4, 2048, 1024] -> qkv proj -> 16-head attention (d=64)
         -> out proj + bias -> [4, 2048, 1024].

Sharding (no collectives): core i handles batch b = i//2 and query-half
half = i%2 (1024 query tokens). Each core computes K/V for its batch's
full 2048-token sequence (duplicated within the pair of cores sharing a
batch) and Q only for its own 1024 tokens. The host rotates the token
axis per core so the core's query tokens are always tokens [0, 1024) of
its input -- attention is permutation-invariant over keys, so K/V token
order does not matter.

Per-core pipeline (everything bf16 on the TensorE, fp32 PSUM accum):
  proj:  Q^T [d, q] / K^T [d, k] head-pairs packed on 128 partitions;
         V [k, d] in 65-wide per-head blocks with a ones column
         (the PV matmul then yields softmax denominators for free).
  attn:  per head: S^T = K @ Q^T -> exp on ScalarE (x0.125 fused, no
         max subtraction; scores are O(1) by construction) -> bf16 P^T
         -> PV accumulation U^T[65, q]; row 64 = denominator. S(k)/exp
         are emitted one k-step ahead of PV(k-1) so the in-order PE
         queue never head-of-line blocks on the exp.
         Tail: one DVE copy [65, q] frees the PSUM slot; the pair tile
         [128, q] bf16 is filled off the PSUM chain (GpSimd lane-local
         for rows 0:64, SBUF-to-SBUF DMA for the cross-partition odd
         half). The slow fp16 softmax reciprocal is split into 4
         [1, 256] chunks drained one per k-step of the NEXT head, so
         the in-order DVE queue never carries a block the PE could
         stall behind (via the mm_psum bank rotation). Normalize =
         ones-row broadcast matmul + DVE multiply straight against the
         PSUM broadcast (no SBUF staging), one head late.
  out:   head-pair packed: stationary [128(d of 2 heads), 128(q)] at
         K=128 keeps the PE at full clock -- K=64 accumulation chains
         measured ~2x slower (p-state/row-group effect). Pass A (pairs
         0-3 + bias) fills PE gaps during the attention tail; pass B
         (pairs 7,4,5,6) accumulates onto resident partials -> out.

The two halves' projections and attention phases are arranged so the
PE always has matmul work while the ScalarE grinds through exp()
(~285us of ACT vs ~465us of PE busy; the kernel is PE-bound).
"""

import numpy as np
import ml_dtypes

B = 4
N = 2048
DIM = 1024
HEADS = 16
DH = 64
NQ = 1024  # query tokens per core
NCORES = 8
NPAIR = HEADS // 2

_CACHE = {}


def _build_nc():
    from contextlib import ExitStack

    import concourse.bass as bass
    import concourse.mybir as mybir
    import concourse.tile as tile
    from concourse import bacc

    f32 = mybir.dt.float32
    bf16 = mybir.dt.bfloat16
    f16 = mybir.dt.float16
    EXP = mybir.ActivationFunctionType.Exp

    nc = bacc.Bacc("TRN2", target_bir_lowering=False, debug=False,
                   num_devices=NCORES)

    xt_d = nc.dram_tensor("xt", [DIM, N], bf16, kind="ExternalInput")
    wqkv_d = nc.dram_tensor("wqkv", [DIM, 3 * DIM], bf16, kind="ExternalInput")
    wo_d = nc.dram_tensor("wo", [HEADS, DH, DIM], bf16, kind="ExternalInput")
    brow_d = nc.dram_tensor("brow", [1, DIM], bf16, kind="ExternalInput")
    out_d = nc.dram_tensor("out", [NQ, DIM], f32, kind="ExternalOutput")

    with tile.TileContext(nc) as tc, ExitStack() as top:
        const_pool = top.enter_context(tc.tile_pool(name="const", bufs=1))
        mm_psum = top.enter_context(tc.tile_pool(name="mmps", bufs=2, space="PSUM"))
        sp_psum = top.enter_context(tc.tile_pool(name="spps", bufs=2, space="PSUM"))
        u_psum = top.enter_context(tc.tile_pool(name="ups", bufs=1, space="PSUM"))
        es_pool = top.enter_context(tc.tile_pool(name="es", bufs=6))
        dn_pool = top.enter_context(tc.tile_pool(name="dn", bufs=4))
        rec_pool = top.enter_context(tc.tile_pool(name="rec", bufs=4))
        bc_pool = top.enter_context(tc.tile_pool(name="bc", bufs=6))
        uraw_a = top.enter_context(tc.tile_pool(name="uraw_a", bufs=1))

        brow_t = const_pool.tile([1, DIM], bf16, tag="brow", name="brow")
        nc.sync.dma_start(brow_t[:], brow_d.ap()[:])
        ones_t = const_pool.tile([1, 128], bf16, tag="ones", name="ones")
        nc.gpsimd.memset(ones_t[:], 1.0)
        # all-ones f16 readable at partition base 64 (row 64) so the
        # denom-broadcast matmul's operands share a base partition
        ones_bc = const_pool.tile([65, 128], f16, tag="ones_bc",
                                  name="ones_bc")
        nc.gpsimd.memset(ones_bc[:], 1.0)

        pairU = [None] * NPAIR  # [128, NQ] bf16: head 2p rows 0:64, 2p+1 rows 64:128
        urs = {}                # per-head [65, NQ] bf16 raw U + denom row
        _recs = {}              # per-head [65, NQ] f16 denom reciprocal (row 64)

        def proj_units(half, w_pool, xtA, xtB, QT, KT, VT):
            """Emission closures, one PSUM-group each.

            Order: all of V, then K/Q alternating per head-pair chunk so
            early head pairs become ready as soon as possible.
            """
            def dma_factory(col0):
                box = [None]
                def dma():
                    if box[0] is None:
                        wb = [w_pool.tile([128, 512], bf16, tag=f"w{fc}",
                                          name=f"w{fc}") for fc in range(8)]
                        for fc in range(8):
                            nc.sync.dma_start(
                                wb[fc][:],
                                wqkv_d.ap()[fc * 128:(fc + 1) * 128,
                                            col0:col0 + 512])
                        box[0] = wb
                    return box[0]
                return dma

            dma_v = dma_factory(2 * DIM + half * 512)
            dma_k = dma_factory(DIM + half * 512)
            dma_q = dma_factory(half * 512)

            def v_unit(mk, dma=dma_v):
                wb = dma()
                xth = xtA if mk < 8 else xtB
                c0 = (mk % 8) * 128
                ps = mm_psum.tile([128, 512], f32, tag="mm", name="mm")
                for fc in range(8):
                    nc.tensor.matmul(
                        ps[:], xth[fc][:, c0:c0 + 128], wb[fc][:],
                        start=(fc == 0), stop=(fc == 7))
                nc.vector.tensor_copy(
                    VT[mk][:, :, 0:64],
                    ps[:].rearrange("p (h d) -> p h d", d=64))
                nc.gpsimd.memset(VT[mk][:, :, 64:65], 1.0)

            def qk_unit(dma, dest, m4, t):
                wb = dma()
                xth = xtA if t < 2 else xtB
                c0 = (t % 2) * 512
                ps = mm_psum.tile([128, 512], f32, tag="mm", name="mm")
                for fc in range(8):
                    nc.tensor.matmul(
                        ps[:], wb[fc][:, m4 * 128:(m4 + 1) * 128],
                        xth[fc][:, c0:c0 + 512],
                        start=(fc == 0), stop=(fc == 7))
                nc.vector.tensor_copy(
                    dest[m4][:, t * 512:(t + 1) * 512], ps[:])

            units = [lambda mk=mk: v_unit(mk) for mk in range(16)]
            for m4 in range(4):
                for t in range(4):
                    units.append(lambda m4=m4, t=t: qk_unit(dma_k, KT, m4, t))
                for t in range(2):
                    units.append(lambda m4=m4, t=t: qk_unit(dma_q, QT, m4, t))
            return units, (dma_v, dma_k, dma_q)

        def emit_norm(p):
            """Normalize pair p's raw U rows by softmax denominators."""
            for par in range(2):
                h = 2 * p + par
                r0 = par * 64
                rec = _recs.pop(h)
                for qc in range(2):
                    bc = mm_psum.tile([128, 512], f32, tag="mm", name="bc")
                    nc.tensor.matmul(
                        bc[:], ones_bc[64:65, :],
                        rec[64:65, qc * 512:(qc + 1) * 512],
                        start=True, stop=True)
                    # multiply straight against the PSUM broadcast (same
                    # base partition on both inputs); no SBUF staging
                    nc.vector.tensor_mul(
                        pairU[p][r0:r0 + 64, qc * 512:(qc + 1) * 512],
                        pairU[p][r0:r0 + 64, qc * 512:(qc + 1) * 512],
                        bc[r0:r0 + 64, :])

        def emit_attn(heads, QTs, KTs, VTs, pairU_pool, fillers,
                      hooks=None):
            """Attention for the given heads; filler units spread across
            k-steps. Normalize for pair p is emitted after head 2p+3."""
            fillers = list(fillers)
            nfill = len(fillers)
            pending_dve = state.setdefault("pending_dve", [])
            steps = len(heads) * 16
            done = 0
            for hh_i, h in enumerate(heads):
                if hooks and h in hooks:
                    hooks[h]()
                half = h // 8
                hh = h % 8
                par = h % 2
                p = h // 2
                QT, KT, VT = QTs[half], KTs[half], VTs[half]
                pair = hh // 2
                hb = (hh % 2) * 64
                Ups = u_psum.tile([65, 2, 512], f32, tag="up", name="up")
                ess = [None, None]
                drain_dve = list(pending_dve)
                pending_dve.clear()
                # S(k)/exp(k) are emitted one step ahead of PV(k-1) so
                # the PE queue never head-of-line blocks on the exp and
                # the new head's first PV gets PSUM-release slack.
                for k in range(16):
                    sp = sp_psum.tile([128, 2, 512], f32, tag="sp", name="sp")
                    for qc in range(2):
                        nc.tensor.matmul(
                            sp[:, qc, :],
                            KT[pair][hb:hb + 64, k * 128:(k + 1) * 128],
                            QT[pair][hb:hb + 64, qc * 512:(qc + 1) * 512],
                            start=True, stop=True)
                    es = es_pool.tile([128, 2, 512], bf16, tag="es", name="es")
                    nc.scalar.activation(es[:], sp[:], EXP, scale=0.125)
                    ess[k % 2] = es
                    if k >= 1:
                        for qc in range(2):
                            nc.tensor.matmul(
                                Ups[:, qc, :],
                                VT[k - 1][:, hh, :],
                                ess[(k - 1) % 2][:, qc, :],
                                start=(k == 1), stop=False)
                    if drain_dve and k % 2 == 1:
                        drain_dve.pop(0)()
                    done += 1
                    while fillers and (nfill - len(fillers)) * steps < done * nfill:
                        fillers.pop(0)()
                for qc in range(2):
                    nc.tensor.matmul(
                        Ups[:, qc, :], VT[15][:, hh, :], ess[15 % 2][:, qc, :],
                        start=False, stop=True)
                # free the PSUM slot fast: U rows on DVE, denom row on
                # GpSimd (parallel engines)
                if par == 0:
                    pairU[p] = pairU_pool(p).tile(
                        [128, NQ], bf16, tag=f"pu{p}", name=f"pu{p}")
                # one fast DVE copy frees the PSUM slot; the pair-packed
                # re-copy runs off the PSUM chain (GpSimd lane-local for
                # the even half, DMA for the cross-partition odd half)
                ur = dn_pool.tile([65, NQ], bf16, tag="ur", name=f"ur{h}")
                urs[h] = ur
                nc.vector.tensor_copy(
                    ur[:].rearrange("p (a b) -> p a b", a=2),
                    Ups[:, :, :])
                if par == 0:
                    nc.gpsimd.tensor_copy(pairU[p][0:64, :], ur[0:64, :])
                else:
                    nc.sync.dma_start(pairU[p][64:128, :], ur[0:64, :])
                # the slow reciprocal is queued in ~1.6us chunks drained
                # one per k-step of the NEXT head, so the in-order DVE
                # queue never carries a block the PE could stall behind
                rec = rec_pool.tile([65, NQ], f16, tag="rec", name="rec")
                _recs[h] = rec

                def recip_chunk(ci, rec=rec, ur=ur):
                    with nc.allow_low_precision(reason="softmax recip f16"):
                        nc.vector.reciprocal(
                            rec[64:65, ci * 256:(ci + 1) * 256],
                            ur[64:65, ci * 256:(ci + 1) * 256])
                for ci in range(4):
                    pending_dve.append(lambda ci=ci, f=recip_chunk: f(ci))
                # pair norm, emitted one head late so the recip chain is
                # long done before the PE broadcast matmul queues
                if par == 1 and p > 0:
                    emit_norm(p - 1)
            for f in fillers:
                f()

        # ---------------- emission ----------------
        if True:
            xt_pool = tc.alloc_tile_pool(name="xt", bufs=1)
            w_pool = tc.alloc_tile_pool(name="w", bufs=3)
            xtA = [xt_pool.tile([128, NQ], bf16, tag=f"xa{i}", name=f"xa{i}")
                   for i in range(8)]
            xtB = [xt_pool.tile([128, NQ], bf16, tag=f"xb{i}", name=f"xb{i}")
                   for i in range(8)]

            qkv0 = tc.alloc_tile_pool(name="qkv0", bufs=1)
            QT0 = [qkv0.tile([128, NQ], bf16, tag=f"q{m}", name=f"q0{m}")
                   for m in range(4)]
            KT0 = [qkv0.tile([128, N], bf16, tag=f"k{m}", name=f"k0{m}")
                   for m in range(4)]
            VT0 = [qkv0.tile([128, 8, 65], bf16, tag=f"v{mk}", name=f"v0{mk}")
                   for mk in range(16)]
            p0_units, p0_dmas = proj_units(0, w_pool, xtA, xtB, QT0, KT0, VT0)

            # DMA order tuned for earliest first matmul: wk0 block plus
            # the first 512 token columns of xt half A unblock the K t=0
            # units at ~2MB of input traffic; the rest streams behind.
            p0_dmas[1]()  # wk half 0
            for i in range(8):
                nc.sync.dma_start(xtA[i][:, 0:512],
                                  xt_d.ap()[i * 128:(i + 1) * 128, 0:512])
            p0_dmas[0]()  # wv half 0
            for i in range(8):
                nc.sync.dma_start(xtA[i][:, 512:NQ],
                                  xt_d.ap()[i * 128:(i + 1) * 128, 512:NQ])
            for i in range(8):
                nc.sync.dma_start(xtB[i][:], xt_d.ap()[i * 128:(i + 1) * 128, NQ:N])

            # head-0/1 prerequisites serially (K pair-0 / V / Q pair-0),
            # resequenced to match DMA arrival order
            u = p0_units
            # K(m4, t=0) units need only wk + the first xtA half: they
            # cover the PE from ~12us while the rest of xtA/wv stream in
            prefix = ([u[16], u[22], u[28], u[34]] + u[0:4] + [u[17]] +
                      u[4:8] + [u[18], u[19], u[20], u[21]] + u[8:16])
            for c in prefix:
                c()
            p0_rest = [u[i] for i in range(22, 40) if i not in (22, 28, 34)]

            qkv1 = tc.alloc_tile_pool(name="qkv1", bufs=1, side="right")
            QT1 = [qkv1.tile([128, NQ], bf16, tag=f"q{m}", name=f"q1{m}")
                   for m in range(4)]
            KT1 = [qkv1.tile([128, N], bf16, tag=f"k{m}", name=f"k1{m}")
                   for m in range(4)]
            VT1 = [qkv1.tile([128, 8, 65], bf16, tag=f"v{mk}", name=f"v1{mk}")
                   for mk in range(16)]
            p1_units, p1_dmas = proj_units(1, w_pool, xtA, xtB, QT1, KT1, VT1)

            state = {}

            def setup_b():
                qkv0.release()
                state["uraw_b"] = tc.alloc_tile_pool(name="uraw_b", bufs=1,
                                                     side="right")
                wo_pool = tc.alloc_tile_pool(name="wo", bufs=1, side="right")
                state["wo_pool"] = wo_pool
                WOP = [wo_pool.tile([128, DIM], bf16, tag=f"wo{p}",
                                    name=f"wo{p}") for p in range(NPAIR)]
                state["WOP"] = WOP
                for p in range(NPAIR):
                    nc.sync.dma_start(WOP[p][0:64, :], wo_d.ap()[2 * p])
                    nc.sync.dma_start(WOP[p][64:128, :], wo_d.ap()[2 * p + 1])

            def setup_c():
                # xt and the w-block tiles are dead once proj(1) is done
                w_pool.release()
                xt_pool.release()
                state["st_pool"] = tc.alloc_tile_pool(name="st", bufs=2)
                state["FIN"] = [
                    state["st_pool"].tile([128, DIM], f32, tag=f"fin{qf}",
                                          name=f"fin{qf}", bufs=1)
                    for qf in range(8)]

            # pass A unit: pairs 0-3 + bias for one qf -> resident FIN tile
            def passA(qf):
                WOP = state["WOP"]
                fin = state["FIN"][qf]
                for of in range(2):
                    ps = mm_psum.tile([128, 512], f32, tag="mm", name="mm")
                    for p in range(4):
                        nc.tensor.matmul(
                            ps[:],
                            pairU[p][:, qf * 128:(qf + 1) * 128],
                            WOP[p][:, of * 512:(of + 1) * 512],
                            start=(p == 0), stop=False)
                    nc.tensor.matmul(
                        ps[:], ones_t[:, 0:128],
                        brow_t[:, of * 512:(of + 1) * 512],
                        start=False, stop=True)
                    nc.vector.tensor_copy(fin[:, of * 512:(of + 1) * 512],
                                          ps[:])

            pairU_pool = lambda p: uraw_a if p < 4 else state["uraw_b"]

            # heads 0-9: remaining proj0 + all proj1 units fill PE gaps
            emit_attn(range(0, 10), [QT0, QT1], [KT0, KT1], [VT0, VT1],
                      pairU_pool, p0_rest + p1_units[:34],
                      hooks={8: setup_b})
            # heads 10-15: pass A units fill PE gaps; reserve 3 pass A
            # units to cover the tail norm chain of pair 7
            tail_fill = [lambda qf=qf: passA(qf) for qf in range(8)]
            emit_attn(range(10, 16), [QT0, QT1], [KT0, KT1], [VT0, VT1],
                      pairU_pool,
                      p1_units[34:] + tail_fill[:5],
                      hooks={10: setup_c})
            # interleave head-15's recip chunks with the reserved pass A
            # units so the PE is never exposed to the norm(7) DVE chain
            pend = list(state.get("pending_dve", []))
            state["pending_dve"] = []
            tf = list(tail_fill[5:])
            while pend or tf:
                if tf:
                    tf.pop(0)()
                if pend:
                    pend.pop(0)()
            emit_norm(7)

            # pass B: pairs 7,4,5,6 onto the resident partials -> out.
            # pair 7 (the norm-gated pair) goes FIRST in each accumulation
            # group so the PE pays its normalize wait once, then streams.
            for qf in range(8):
                fin = state["FIN"][qf]
                for of in range(2):
                    ps = mm_psum.tile([128, 512], f32, tag="mm", name="mm")
                    for pi, p in enumerate([7, 4, 5, 6]):
                        nc.tensor.matmul(
                            ps[:],
                            pairU[p][:, qf * 128:(qf + 1) * 128],
                            state["WOP"][p][:, of * 512:(of + 1) * 512],
                            start=(pi == 0), stop=(pi == 3))
                    nc.vector.tensor_add(
                        fin[:, of * 512:(of + 1) * 512],
                        fin[:, of * 512:(of + 1) * 512], ps[:])
                    nc.sync.dma_start(
                        out_d.ap()[qf * 128:(qf + 1) * 128,
                                   of * 512:(of + 1) * 512],
                        fin[:, of * 512:(of + 1) * 512])

            state["st_pool"].release()
            state["wo_pool"].release()
            state["uraw_b"].release()
            qkv1.release()

    nc.compile()
    return nc


def _get_nc():
    if "nc" not in _CACHE:
        _CACHE["nc"] = _build_nc()
    return _CACHE["nc"]


def _make_in_maps(x, w_qkv, w_out, b_out):
    bf = ml_dtypes.bfloat16
    wo = np.ascontiguousarray(w_out.reshape(HEADS, DH, DIM)).astype(bf)
    brow = np.asarray(b_out, np.float32).reshape(1, DIM).astype(bf)
    wqkv = np.ascontiguousarray(w_qkv, np.float32).astype(bf)
    in_maps = []
    for i in range(NCORES):
        b, half = i // 2, i % 2
        xt = np.asarray(x[b], np.float32).T.astype(bf)  # [DIM, N]
        if half:
            xt = np.concatenate([xt[:, NQ:], xt[:, :NQ]], axis=1)
        in_maps.append({
            "xt": np.ascontiguousarray(xt),
            "wqkv": wqkv,
            "wo": wo,
            "brow": brow,
        })
    return in_maps


def _assemble(results):
    out = np.empty((B, N, DIM), np.float32)
    for i in range(NCORES):
        b, half = i // 2, i % 2
        out[b, half * NQ:(half + 1) * NQ, :] = results[i]["out"]
    return out


def run(x, w_qkv, w_out, b_out, trace=False):
    """Run the kernel; returns (output, BassKernelResults)."""
    from concourse.bass_utils import run_bass_kernel_spmd
    nc = _get_nc()
    in_maps = _make_in_maps(x, w_qkv, w_out, b_out)
    res = run_bass_kernel_spmd(nc, in_maps, core_ids=list(range(NCORES)),
                               trace=trace)
    return _assemble(res.results), res


def kernel(x, w_qkv, w_out, b_out):
    out, _ = run(x, w_qkv, w_out, b_out, trace=False)
    return out
